# revision 21
# baseline (speedup 1.0000x reference)
"""Trainium2 Bass kernel for nn_AttentionThresholdCoupler.

Math notes (derived from the reference):
  - The cross-attention has query/key sequence length 1, so softmax over the
    single key is exactly 1.0: attention weights gw/cw are all-ones and the
    attention blocks collapse to affine maps.
  - Path A (ga): LN(x + c1) with c1 a constant vector, then FFN residual.
  - Path B (ca): LN(x @ A2 + c2) with A2 = wv2.T @ wo2.T, then FFN residual.
  - unified = tanh(ga @ Wa + ca @ Wb + bi).
All per-row ops act on K=8 features; rows are B*S = 524288, data-parallel
across 8 cores (65536 rows each).

Device layout: "blk" layout puts K on partitions: a tile [128, 512] holds 16
row-groups x 8 features on partitions, 512 row-slots on the free dim.  PE
transposes [128,128] chunks convert between the HBM-natural layout (rows on
partitions) and blk.  All small matmuls become block-diagonal [128,128]
matmuls (np.kron(I16, M)).
"""

import sys

if '/opt/trn_rl_repo' not in sys.path:
    sys.path.insert(0, '/opt/trn_rl_repo')

import numpy as np

import concourse.bass as bass
import concourse.mybir as mybir
import concourse.tile as tile
from concourse.vector_clock import ScopedClock, VectorClock

K = 8
B = 256
S = 2048
N = B * S
NCORES = 8
RPC = N // NCORES          # rows per core = 65536
F = 512                    # free dim per X-tile (one PSUM bank of fp32)
NT = RPC // (16 * F)       # X-tiles per core = 8
EPS = 1e-5

FP = mybir.dt.float32
AF = mybir.ActivationFunctionType
OP = mybir.AluOpType

# matmul operand mode: "f32" | "f32r" | "bf16"
MM_MODE = "f32"
# rstd mode: "ars" (Abs_reciprocal_sqrt, 1 op) | "lnexp" (Ln+Exp, 2 ops)
RSTD_MODE = "lnexp"


def _patch_tile_drain():
    """walrus in this container rejects >1 sync waits on the Tile exit
    drain; split them across one drain instruction per proc."""
    if getattr(tile.TileContext, '_drain_split_patched', False):
        return

    def _drain_and_barrier_split(self, tick_clock, wait_clock):
        nc = self.nc
        vclock = tick_clock.global_clock
        n = len(vclock)
        for i in range(n):
            t = vclock[i]
            if t > 0:
                partial = VectorClock([t if j == i else 0 for j in range(n)])
                d = nc.sync.drain()
                wait_clock.add_sem_waits(d.ins, ScopedClock({None: partial}))
        nc.sync.drain()
        nc.all_engine_barrier()
        assert self.sems is not None
        popped = nc._tile_sem_poison_stack.pop()
        assert popped is self._sem_poison
        nc.clear_and_free_semaphores(list(self.sems.allocated().values()))
        nc.all_engine_barrier()

    tile.TileContext._drain_and_barrier = _drain_and_barrier_split
    tile.TileContext._drain_split_patched = True


_WAIT_LIMIT = 1


def _split_multi_waits(nc):
    """walrus here rejects instructions carrying more than ~1 sync wait.
    Move excess waits onto same-engine NOPs inserted before the holder."""
    cnt = [0]
    for f in nc.m.functions:
        for blk in f.blocks:
            insts = blk.instructions
            out = []
            for inst in insts:
                si = inst.sync_info
                if si is not None and si.on_wait and len(si.on_wait) > _WAIT_LIMIT:
                    waits = list(si.on_wait)
                    keep = waits[-_WAIT_LIMIT:]
                    spill = waits[:-_WAIT_LIMIT]
                    for w in spill:
                        nop = mybir.InstNoOp(
                            name=f"waitnop-{cnt[0]}", ins=[], outs=[])
                        cnt[0] += 1
                        nop.engine = inst.engine
                        nop.sync_info = mybir.SyncInfo(
                            on_wait=[w], on_update=[])
                        out.append(nop)
                    inst.sync_info = mybir.SyncInfo(
                        on_wait=keep, on_update=list(si.on_update or []))
                out.append(inst)
            blk.instructions = out


def _mm_dt():
    return {"f32": mybir.dt.float32,
            "f32r": mybir.dt.float32r,
            "bf16": mybir.dt.bfloat16}[MM_MODE]


def _mm(ap):
    """Cast an f32 AP to the matmul operand dtype (bitcast for f32r)."""
    if MM_MODE == "f32r":
        return ap.bitcast(mybir.dt.float32r)
    return ap


_NC_CACHE = {}


def _build_nc():
    """Build the Bass module (shape-only; weights arrive as inputs)."""
    key = (MM_MODE, RSTD_MODE)
    if key in _NC_CACHE:
        return _NC_CACHE[key]
    _patch_tile_drain()
    nc = bass.Bass()
    wdt = mybir.dt.bfloat16 if MM_MODE == "bf16" else FP
    ddt = mybir.dt.bfloat16 if MM_MODE == "bf16" else FP

    x_in = nc.declare_dram_parameter("x", [RPC, K], FP, isOutput=False)
    # packed lhsT weights: 12 main + 8 stat-spread + 8 rstd-bcast + identity
    NW = 29
    wpack = nc.declare_dram_parameter("wpack", [128, NW * 128], FP, isOutput=False)
    vpack = nc.declare_dram_parameter("vpack", [128, 8], FP, isOutput=False)
    biu = nc.declare_dram_parameter("biu", [128, 1], FP, isOutput=False)
    natb = nc.declare_dram_parameter("natb", [3 * F], FP, isOutput=False)
    ga_out = nc.declare_dram_parameter("ga", [RPC, K], FP, isOutput=True)
    ca_out = nc.declare_dram_parameter("ca", [RPC, K], FP, isOutput=True)
    un_out = nc.declare_dram_parameter("un", [RPC, K], FP, isOutput=True)

    with tile.TileContext(nc) as tc:
        with (
            tc.tile_pool(name="consts", bufs=1) as cp,
            tc.tile_pool(name="bigio", bufs=1) as bigio,
            tc.tile_pool(name="persist", bufs=1) as pp,
        ):
            # ---- constants ----
            wsb_f32 = cp.tile([128, NW * 128], FP)
            nc.sync.dma_start(out=wsb_f32[:, :], in_=wpack[:, :])
            if wdt != FP:
                wsb = cp.tile([128, NW * 128], wdt)
                nc.vector.tensor_copy(wsb[:, :], wsb_f32[:, :])
            else:
                wsb = wsb_f32

            def W(i):  # i-th [128,128] lhsT
                return wsb[:, i * 128:(i + 1) * 128]

            (L_MA, L_MB, L_W1A_t, L_W1A_b, L_W1B_t, L_W1B_b,
             L_W2A_t, L_W2A_b, L_W2B_t, L_W2B_b, L_UA, L_UB) = \
                [W(i) for i in range(12)]
            L_SP = [W(12 + i) for i in range(NT)]
            L_BCt = [W(20 + i) for i in range(NT)]
            # identity for transposes stays f32 always
            L_IDF = wsb_f32[:, 28 * 128:29 * 128]

            vsb = cp.tile([128, 8], FP)
            nc.sync.dma_start(out=vsb[:, :], in_=vpack[:, :])
            cc1_v = vsb[:, 0:1]
            wB_v = vsb[:, 1:2]
            g1_v = vsb[:, 2:3]
            g2_v = vsb[:, 3:4]
            b1At_v = vsb[:, 4:5]
            b1Ab_v = vsb[:, 5:6]
            # packed: 4=b1A top, 5=b1A bot, 6=b1B top, 7=b1B bot... need biU too
            b1Bt_v = vsb[:, 6:7]
            b1Bb_v = vsb[:, 7:8]
            vsb2 = cp.tile([128, 1], FP)
            nc.sync.dma_start(out=vsb2[:, :], in_=biu[:, :])
            eps_t = cp.tile([128, 1], FP)
            nc.vector.memset(eps_t[:, :], EPS)
            zero_t = cp.tile([128, 1], FP)
            nc.vector.memset(zero_t[:, :], 0.0)

            natb_sb = cp.tile([128, 3 * F], FP)
            natb_ap = natb[:]
            natb_bcast = bass.AP(
                tensor=natb_ap.tensor, offset=natb_ap.offset,
                ap=[[0, 128]] + [list(d) for d in natb_ap.ap])
            nc.sync.dma_start(out=natb_sb[:, :], in_=natb_bcast)

            # ---- big IO tiles ----
            X_nat = bigio.tile([128, RPC * K // 128], FP)          # [128, 4096]
            nc.sync.dma_start(
                out=X_nat[:, :],
                in_=x_in.rearrange("(p r) k -> p (r k)", p=128))
            nat_all = bigio.tile([128, 3 * RPC * K // 128], FP)    # [128, 12288]
            NATW = RPC * K // 128                                   # 4096

            sh_all = pp.tile([128, 2 * NT * F], FP)                # [128, 8192]

            def shA(t):
                return sh_all[:, t * F:(t + 1) * F]

            def shB(t):
                return sh_all[:, (NT + t) * F:(NT + t + 1) * F]

            rstdA = pp.tile([128, F], FP)
            rstdB = pp.tile([128, F], FP)

            # ================= sweep 1 =================
            with (
                tc.tile_pool(name="ps_ti", bufs=2, space="PSUM") as ps_ti,
                tc.tile_pool(name="ps_sh", bufs=2, space="PSUM") as ps_sh,
                tc.tile_pool(name="ps_st", bufs=1, space="PSUM") as ps_st,
                tc.tile_pool(name="s1", bufs=3) as s1,
            ):
                st_A = ps_st.tile([128, F], FP, tag="stA")
                st_B = ps_st.tile([128, F], FP, tag="stB")
                for t in range(NT):
                    pi = ps_ti.tile([128, F], FP, tag="pi")
                    for c in range(4):
                        nc.tensor.transpose(
                            pi[:, c * 128:(c + 1) * 128],
                            X_nat[:, (4 * t + c) * 128:(4 * t + c + 1) * 128],
                            L_IDF)
                    xblk = s1.tile([128, F], ddt, tag="xblk")
                    nc.scalar.copy(xblk[:, :], pi[:, :])
                    psA = ps_sh.tile([128, F], FP, tag="ps")
                    nc.tensor.matmul(psA[:, :], _mm(L_MA), _mm(xblk[:, :]))
                    nc.vector.tensor_scalar_add(shA(t), psA[:, :], cc1_v)
                    sqA = s1.tile([128, F], ddt, tag="sqA")
                    nc.scalar.square(sqA[:, :], shA(t))
                    nc.tensor.matmul(st_A[:, :], _mm(L_SP[t]), _mm(sqA[:, :]),
                                     start=(t == 0), stop=(t == NT - 1))
                    psB = ps_sh.tile([128, F], FP, tag="ps")
                    nc.tensor.matmul(psB[:, :], _mm(L_MB), _mm(xblk[:, :]))
                    nc.vector.tensor_scalar_add(shB(t), psB[:, :], wB_v)
                    sqB = s1.tile([128, F], ddt, tag="sqB")
                    nc.scalar.square(sqB[:, :], shB(t))
                    nc.tensor.matmul(st_B[:, :], _mm(L_SP[t]), _mm(sqB[:, :]),
                                     start=(t == 0), stop=(t == NT - 1))

                # rstd = (sumsq/8 + eps)^-1/2
                if RSTD_MODE == "ars":
                    nc.scalar.activation(rstdA[:, :], st_A[:, :],
                                         AF.Abs_reciprocal_sqrt,
                                         bias=eps_t[:, 0:1], scale=1.0 / K)
                    nc.scalar.activation(rstdB[:, :], st_B[:, :],
                                         AF.Abs_reciprocal_sqrt,
                                         bias=eps_t[:, 0:1], scale=1.0 / K)
                else:
                    lt = s1.tile([128, F], FP, tag="lntmp")
                    nc.scalar.activation(lt[:, :], st_A[:, :], AF.Ln,
                                         bias=eps_t[:, 0:1], scale=1.0 / K)
                    nc.scalar.activation(rstdA[:, :], lt[:, :], AF.Exp,
                                         bias=zero_t[:, 0:1], scale=-0.5)
                    lt2 = s1.tile([128, F], FP, tag="lntmp")
                    nc.scalar.activation(lt2[:, :], st_B[:, :], AF.Ln,
                                         bias=eps_t[:, 0:1], scale=1.0 / K)
                    nc.scalar.activation(rstdB[:, :], lt2[:, :], AF.Exp,
                                         bias=zero_t[:, 0:1], scale=-0.5)

            # ================= sweep 2 =================
            with (
                tc.tile_pool(name="ps_bc", bufs=1, space="PSUM") as ps_bc,
                tc.tile_pool(name="ps_h", bufs=2, space="PSUM") as ps_h,
                tc.tile_pool(name="ps_f", bufs=2, space="PSUM") as ps_f,
                tc.tile_pool(name="ps_o", bufs=1, space="PSUM") as ps_o,
                tc.tile_pool(name="s2", bufs=2) as s2,
                tc.tile_pool(name="s2h", bufs=4) as s2h,
            ):
                for t in range(NT):
                    po = ps_o.tile([128, 3 * F], FP, tag="po")
                    for path in ("A", "B"):
                        rstd = rstdA if path == "A" else rstdB
                        sh = shA(t) if path == "A" else shB(t)
                        g_v = g1_v if path == "A" else g2_v
                        L1t = L_W1A_t if path == "A" else L_W1B_t
                        L1b = L_W1A_b if path == "A" else L_W1B_b
                        L2t = L_W2A_t if path == "A" else L_W2B_t
                        L2b = L_W2A_b if path == "A" else L_W2B_b
                        bt_v = b1At_v if path == "A" else b1Bt_v
                        bb_v = b1Ab_v if path == "A" else b1Bb_v

                        bc = ps_bc.tile([128, F], FP, tag="bc")
                        nc.tensor.matmul(bc[:, :], _mm(L_BCt[t]),
                                         _mm(rstd[:, :]))
                        z = s2.tile([128, F], ddt, tag="z" + path)
                        nc.vector.tensor_mul(z[:, :], sh, bc[:, :])
                        ph1 = ps_h.tile([128, F], FP, tag="ph")
                        nc.tensor.matmul(ph1[:, :], _mm(L1t), _mm(z[:, :]))
                        h1 = s2h.tile([128, F], ddt, tag="h")
                        nc.scalar.activation(h1[:, :], ph1[:, :], AF.Relu,
                                             bias=bt_v, scale=1.0)
                        ph2 = ps_h.tile([128, F], FP, tag="ph")
                        nc.tensor.matmul(ph2[:, :], _mm(L1b), _mm(z[:, :]))
                        h2 = s2h.tile([128, F], ddt, tag="h")
                        nc.scalar.activation(h2[:, :], ph2[:, :], AF.Relu,
                                             bias=bb_v, scale=1.0)
                        pf = ps_f.tile([128, F], FP, tag="pf")
                        nc.tensor.matmul(pf[:, :], _mm(L2t), _mm(h1[:, :]),
                                         start=True, stop=False)
                        nc.tensor.matmul(pf[:, :], _mm(L2b), _mm(h2[:, :]),
                                         start=False, stop=True)
                        res = s2.tile([128, F], ddt, tag="res" + path)
                        nc.vector.scalar_tensor_tensor(
                            res[:, :], in0=z[:, :], scalar=g_v, in1=pf[:, :],
                            op0=OP.mult, op1=OP.add)
                        if path == "A":
                            resA = res
                        else:
                            resB = res

                    pu = ps_f.tile([128, F], FP, tag="pf")
                    nc.tensor.matmul(pu[:, :], _mm(L_UA), _mm(resA[:, :]),
                                     start=True, stop=False)
                    nc.tensor.matmul(pu[:, :], _mm(L_UB), _mm(resB[:, :]),
                                     start=False, stop=True)
                    ublk = s2.tile([128, F], FP, tag="ublk")
                    nc.scalar.activation(ublk[:, :], pu[:, :], AF.Tanh,
                                         bias=vsb2[:, 0:1], scale=1.0)

                    for c in range(4):
                        sl = slice(c * 128, (c + 1) * 128)
                        nc.tensor.transpose(po[:, 0 * F + c * 128:0 * F + (c + 1) * 128],
                                            resA[:, sl], L_IDF)
                        nc.tensor.transpose(po[:, 1 * F + c * 128:1 * F + (c + 1) * 128],
                                            resB[:, sl], L_IDF)
                        nc.tensor.transpose(po[:, 2 * F + c * 128:2 * F + (c + 1) * 128],
                                            ublk[:, sl], L_IDF)

                    nc.vector.tensor_add(
                        nat_all[:, :].rearrange("p (r x) -> p r x", r=3)[:, :, t * F:(t + 1) * F],
                        po[:, :].rearrange("p (r x) -> p r x", r=3),
                        natb_sb[:, :].rearrange("p (r x) -> p r x", r=3))

            nc.sync.dma_start(
                out=ga_out.rearrange("(p r) k -> p (r k)", p=128),
                in_=nat_all[:, 0 * NATW:1 * NATW])
            nc.sync.dma_start(
                out=ca_out.rearrange("(p r) k -> p (r k)", p=128),
                in_=nat_all[:, 1 * NATW:2 * NATW])
            nc.sync.dma_start(
                out=un_out.rearrange("(p r) k -> p (r k)", p=128),
                in_=nat_all[:, 2 * NATW:3 * NATW])

    _split_multi_waits(nc)
    _NC_CACHE[key] = nc
    return nc


def _fold_constants(coral_taus, params):
    """Host-side constant folding in float64."""
    p = {k: {kk: np.asarray(vv, np.float64) for kk, vv in v.items()}
         if isinstance(v, dict) else np.asarray(v, np.float64)
         for k, v in params.items()}
    taus = np.asarray(coral_taus, np.float64)

    a1, a2 = p['attn1'], p['attn2']
    c1 = (taus @ a1['wv'].T + a1['bv']) @ a1['wo'].T + a1['bo']       # [8]
    A2 = a2['wv'].T @ a2['wo'].T                                       # [8,8] right-mult
    c2v = a2['bv'] @ a2['wo'].T + a2['bo'] + taus                      # [8]

    C = np.eye(K) - np.ones((K, K)) / K
    M_A = C
    M_B = A2 @ C
    cc1 = c1 @ C
    wB = c2v @ C

    def ffn_fold(fp, g, b):
        w1, b1, w2, b2 = fp['w1'], fp['b1'], fp['w2'], fp['b2']
        W1eff = (w1 * g[None, :]).T            # [8,16]: W1eff[k,j] = g[k] w1[j,k]
        b1eff = b @ w1.T + b1                  # [16]
        W2eff = w2.T                           # [16,8]
        bb = b + b2                            # [8]
        return W1eff, b1eff, W2eff, bb

    g1, b1v = p['ln1_g'], p['ln1_b']
    g2, b2v = p['ln2_g'], p['ln2_b']
    W1A, b1A, W2A, bbA = ffn_fold(p['ffn1'], g1, b1v)
    W1B, b1B, W2B, bbB = ffn_fold(p['ffn2'], g2, b2v)

    int_w, int_b = p['int_w'], p['int_b']
    Wa = int_w[:, :K].T                        # [8,8]
    Wb = int_w[:, K:].T
    biU = int_b + bbA @ Wa + bbB @ Wb

    I16 = np.eye(16)

    def kr(M):
        return np.kron(I16, M)

    mats = [kr(M_A), kr(M_B),
            kr(W1A[:, 0:8]), kr(W1A[:, 8:16]),
            kr(W1B[:, 0:8]), kr(W1B[:, 8:16]),
            kr(W2A[0:8, :]), kr(W2A[8:16, :]),
            kr(W2B[0:8, :]), kr(W2B[8:16, :]),
            kr(Wa), kr(Wb)]
    nt = RPC // (16 * F)
    ones16 = np.kron(I16, np.ones((8, 1)))     # [128, 16]
    for t in range(nt):                        # stat spread: sums land at 16t+g
        sp = np.zeros((128, 128))
        sp[:, 16 * t:16 * t + 16] = ones16
        mats.append(sp)
    for t in range(nt):                        # rstd bcast: out (g,k) <- rstd[16t+g]
        bc = np.zeros((128, 128))
        bc[16 * t:16 * t + 16, :] = np.kron(I16, np.ones((1, 8)))
        mats.append(bc)
    mats.append(np.eye(128))                   # identity for transposes
    wpack = np.concatenate(mats, axis=1)

    def t16(v):
        return np.tile(v, 16)

    vpack = np.stack([t16(cc1), t16(wB), t16(g1), t16(g2),
                      t16(b1A[0:8]), t16(b1A[8:16]),
                      t16(b1B[0:8]), t16(b1B[8:16])], axis=1)   # [128, 8]

    natb = np.concatenate([np.tile(bbA, 64), np.tile(bbB, 64),
                           np.zeros(F)])                          # [1536]

    biU_t = t16(biU)                                              # [128]
    return (wpack.astype(np.float32), vpack.astype(np.float32),
            natb.astype(np.float32), biU_t.astype(np.float32))


_RUNNER_CACHE = {}


def _make_runner(nc):
    """Cached jitted shard_map executable over the 8 cores (no donation so
    it can be re-invoked for timing)."""
    if id(nc) in _RUNNER_CACHE:
        return _RUNNER_CACHE[id(nc)]
    import jax
    from jax.experimental.shard_map import shard_map
    from jax.sharding import Mesh, PartitionSpec
    from concourse import bass2jax
    import concourse.mybir as _mybir

    bass2jax.install_neuronx_cc_hook()
    in_names, out_names, out_avals, zero_shapes = [], [], [], []
    for alloc in nc.m.functions[0].allocations:
        if not isinstance(_mybir.MemoryLocationSet, type) or not isinstance(
                alloc, _mybir.MemoryLocationSet):
            continue
        name = alloc.memorylocations[0].name
        pname = (nc.partition_id_tensor.name
                 if nc.partition_id_tensor else None)
        if alloc.kind == "ExternalInput":
            if name != pname:
                in_names.append(name)
        elif alloc.kind == "ExternalOutput":
            out_names.append(name)
            shape = tuple(alloc.tensor_shape)
            dtype = _mybir.dt.np(alloc.dtype)
            out_avals.append(jax.core.ShapedArray(shape, dtype))
            zero_shapes.append((shape, dtype))
    n_params = len(in_names)
    all_names = list(in_names) + list(out_names)
    if nc.partition_id_tensor is not None:
        all_names.append(nc.partition_id_tensor.name)

    def _body(*args):
        operands = list(args)
        if nc.partition_id_tensor is not None:
            operands.append(bass2jax.partition_id_tensor())
        outs = bass2jax._bass_exec_p.bind(
            *operands,
            out_avals=tuple(out_avals),
            in_names=tuple(all_names),
            out_names=tuple(out_names),
            lowering_input_output_aliases=(),
            sim_require_finite=True,
            sim_require_nnan=True,
            nc=nc)
        return tuple(outs)

    devices = jax.devices()[:NCORES]
    mesh = Mesh(np.asarray(devices), ("core",))
    n_args = n_params + len(out_names)
    fn = jax.jit(shard_map(
        _body, mesh=mesh,
        in_specs=(PartitionSpec("core"),) * n_args,
        out_specs=(PartitionSpec("core"),) * len(out_names),
        check_rep=False))
    runner = (fn, in_names, out_names, zero_shapes)
    _RUNNER_CACHE[id(nc)] = runner
    return runner


def _run(nc, in_maps):
    fn, in_names, out_names, zero_shapes = _make_runner(nc)
    ncores = len(in_maps)
    concat_in = [np.concatenate([np.asarray(m[n]) for m in in_maps], axis=0)
                 for n in in_names]
    zeros = [np.zeros((ncores * s[0], *s[1:]), d) for s, d in zero_shapes]
    outs = fn(*concat_in, *zeros)
    results = []
    for c in range(ncores):
        results.append({
            name: np.asarray(outs[i]).reshape(ncores, *zero_shapes[i][0])[c]
            for i, name in enumerate(out_names)})
    return results


def _make_in_maps(gpcm_betas, coral_taus, params):
    x = np.asarray(gpcm_betas, np.float32).reshape(N, K)
    wpack, vpack, natb, biU_t = _fold_constants(coral_taus, params)
    in_maps = []
    for i in range(NCORES):
        shard = np.ascontiguousarray(x[i * RPC:(i + 1) * RPC])
        in_maps.append({"x": shard, "wpack": wpack, "vpack": vpack,
                        "natb": natb, "biu": biU_t.reshape(128, 1)})
    return in_maps


def bench(inputs, reps=20):
    """Min wall time per executable invocation (device exec + dispatch)."""
    import time as _time
    import jax
    nc = _build_nc()
    in_maps = _make_in_maps(inputs["gpcm_betas"], inputs["coral_taus"],
                            inputs["params"])
    fn, in_names, out_names, zero_shapes = _make_runner(nc)
    ncores = len(in_maps)
    concat_in = [np.concatenate([np.asarray(m[n]) for m in in_maps], axis=0)
                 for n in in_names]
    zeros = [np.zeros((ncores * s[0], *s[1:]), d) for s, d in zero_shapes]
    args = [jax.device_put(a) for a in concat_in + zeros]
    best = float("inf")
    for _ in range(reps):
        t0 = _time.perf_counter()
        outs = fn(*args)
        jax.block_until_ready(outs)
        dt = _time.perf_counter() - t0
        best = min(best, dt)
    return best * 1e9


def kernel(gpcm_betas, coral_taus, theta, params):
    nc = _build_nc()
    in_maps = _make_in_maps(gpcm_betas, coral_taus, params)
    results = _run(nc, in_maps)

    ga = np.concatenate([results[i]["ga"] for i in range(NCORES)], axis=0)
    ca = np.concatenate([results[i]["ca"] for i in range(NCORES)], axis=0)
    un = np.concatenate([results[i]["un"] for i in range(NCORES)], axis=0)

    ones = np.ones((N, 1), np.float32)
    return (un.reshape(B, S, K), ga.reshape(B, S, K), ca.reshape(B, S, K),
            ones, ones.copy())


# revision 31
# speedup vs baseline: 1.0121x; 1.0121x over previous
"""Trainium2 Bass kernel for nn_AttentionThresholdCoupler.

Math notes (derived from the reference):
  - The cross-attention has query/key sequence length 1, so softmax over the
    single key is exactly 1.0: attention weights gw/cw are all-ones and the
    attention blocks collapse to affine maps.
  - Path A (ga): LN(x + c1) with c1 a constant vector, then FFN residual.
  - Path B (ca): LN(x @ A2 + c2) with A2 = wv2.T @ wo2.T, then FFN residual.
  - unified = tanh(ga @ Wa + ca @ Wb + bi).
All per-row ops act on K=8 features; rows are B*S = 524288, data-parallel
across 8 cores (65536 rows each).

Device layout: "blk" layout puts K on partitions: a tile [128, 512] holds 16
row-groups x 8 features on partitions, 512 row-slots on the free dim.  PE
transposes [128,128] chunks convert between the HBM-natural layout (rows on
partitions) and blk.  All small matmuls become block-diagonal [128,128]
matmuls (np.kron(I16, M)).
"""

import sys

if '/opt/trn_rl_repo' not in sys.path:
    sys.path.insert(0, '/opt/trn_rl_repo')

import numpy as np

import concourse.bass as bass
import concourse.mybir as mybir
import concourse.tile as tile
from concourse.vector_clock import ScopedClock, VectorClock

K = 8
B = 256
S = 2048
N = B * S
NCORES = 8
RPC = N // NCORES          # rows per core = 65536
F = 512                    # free dim per X-tile (one PSUM bank of fp32)
NT = RPC // (16 * F)       # X-tiles per core = 8
EPS = 1e-5

FP = mybir.dt.float32
AF = mybir.ActivationFunctionType
OP = mybir.AluOpType

# matmul operand mode: "f32" | "f32r" | "bf16"
MM_MODE = "f32r"
# rstd mode: "ars" (Abs_reciprocal_sqrt, 1 op) | "lnexp" (Ln+Exp, 2 ops)
RSTD_MODE = "lnexp"


def _patch_tile_drain():
    """walrus in this container rejects >1 sync waits on the Tile exit
    drain; split them across one drain instruction per proc."""
    if getattr(tile.TileContext, '_drain_split_patched', False):
        return

    def _drain_and_barrier_split(self, tick_clock, wait_clock):
        nc = self.nc
        vclock = tick_clock.global_clock
        n = len(vclock)
        for i in range(n):
            t = vclock[i]
            if t > 0:
                partial = VectorClock([t if j == i else 0 for j in range(n)])
                d = nc.sync.drain()
                wait_clock.add_sem_waits(d.ins, ScopedClock({None: partial}))
        nc.sync.drain()
        nc.all_engine_barrier()
        assert self.sems is not None
        popped = nc._tile_sem_poison_stack.pop()
        assert popped is self._sem_poison
        nc.clear_and_free_semaphores(list(self.sems.allocated().values()))
        nc.all_engine_barrier()

    tile.TileContext._drain_and_barrier = _drain_and_barrier_split
    tile.TileContext._drain_split_patched = True


_WAIT_LIMIT = 1


def _split_multi_waits(nc):
    """walrus here rejects instructions carrying more than ~1 sync wait.
    Move excess waits onto same-engine NOPs inserted before the holder."""
    cnt = [0]
    for f in nc.m.functions:
        for blk in f.blocks:
            insts = blk.instructions
            out = []
            for inst in insts:
                si = inst.sync_info
                if si is not None and si.on_wait and len(si.on_wait) > _WAIT_LIMIT:
                    waits = list(si.on_wait)
                    keep = waits[-_WAIT_LIMIT:]
                    spill = waits[:-_WAIT_LIMIT]
                    for w in spill:
                        nop = mybir.InstNoOp(
                            name=f"waitnop-{cnt[0]}", ins=[], outs=[])
                        cnt[0] += 1
                        nop.engine = inst.engine
                        nop.sync_info = mybir.SyncInfo(
                            on_wait=[w], on_update=[])
                        out.append(nop)
                    inst.sync_info = mybir.SyncInfo(
                        on_wait=keep, on_update=list(si.on_update or []))
                out.append(inst)
            blk.instructions = out


def _mm_dt():
    return {"f32": mybir.dt.float32,
            "f32r": mybir.dt.float32r,
            "bf16": mybir.dt.bfloat16}[MM_MODE]


def _mm(ap):
    """Matmul operand APs are pre-typed via tile dtypes; no-op."""
    return ap


_NC_CACHE = {}


def _build_nc():
    """Build the Bass module (shape-only; weights arrive as inputs)."""
    key = (MM_MODE, RSTD_MODE)
    if key in _NC_CACHE:
        return _NC_CACHE[key]
    _patch_tile_drain()
    nc = bass.Bass()
    wdt = _mm_dt()
    ddt = _mm_dt()
    BF = MM_MODE == "bf16"
    BT = mybir.dt.bfloat16
    xdt = _mm_dt() if MM_MODE != "f32" else FP   # input pipeline dtype
    odt = BT if BF else FP          # output pipeline dtype
    pdt = FP                        # matmul outputs must be fp32

    x_in = nc.declare_dram_parameter("x", [RPC, K], xdt, isOutput=False)
    # (f32r container is np.float32; DMA bytes are unrounded fp32, PE rounds)
    # packed lhsT weights: 12 main + 8 stat-spread + 8 rstd-bcast + identity
    NW = 29
    wpack = nc.declare_dram_parameter("wpack", [128, NW * 128],
                                      _mm_dt() if MM_MODE != "f32" else FP,
                                      isOutput=False)
    vpack = nc.declare_dram_parameter("vpack", [128, 8], FP, isOutput=False)
    biu = nc.declare_dram_parameter("biu", [128, 1], FP, isOutput=False)
    natb = nc.declare_dram_parameter("natb", [3 * F], odt, isOutput=False)
    ga_out = nc.declare_dram_parameter("ga", [RPC, K], odt, isOutput=True)
    ca_out = nc.declare_dram_parameter("ca", [RPC, K], odt, isOutput=True)
    un_out = nc.declare_dram_parameter("un", [RPC, K], odt, isOutput=True)

    with tile.TileContext(nc) as tc:
        with (
            tc.tile_pool(name="consts", bufs=1) as cp,
            tc.tile_pool(name="persist", bufs=1) as pp,
        ):
            # ---- constants ----
            wsb = cp.tile([128, NW * 128], wdt)
            nc.sync.dma_start(out=wsb[:, :], in_=wpack[:, :])
            wsb_f32 = wsb

            def W(i):  # i-th [128,128] lhsT
                return wsb[:, i * 128:(i + 1) * 128]

            (L_MA, L_MB, L_W1A_t, L_W1A_b, L_W1B_t, L_W1B_b,
             L_W2A_t, L_W2A_b, L_W2B_t, L_W2B_b, L_UA, L_UB) = \
                [W(i) for i in range(12)]
            L_SP = [W(12 + i) for i in range(NT)]
            L_BCt = [W(20 + i) for i in range(NT)]
            # f32 identity for the input transposes (X_nat is f32);
            # dtype-matched identity for the output transposes
            L_IDF = wsb_f32[:, 28 * 128:29 * 128]
            L_IDD = wsb[:, 28 * 128:29 * 128]

            vsb = cp.tile([128, 8], FP)
            nc.sync.dma_start(out=vsb[:, :], in_=vpack[:, :])
            cc1_v = vsb[:, 0:1]
            wB_v = vsb[:, 1:2]
            g1_v = vsb[:, 2:3]
            g2_v = vsb[:, 3:4]
            b1At_v = vsb[:, 4:5]
            b1Ab_v = vsb[:, 5:6]
            # packed: 4=b1A top, 5=b1A bot, 6=b1B top, 7=b1B bot... need biU too
            b1Bt_v = vsb[:, 6:7]
            b1Bb_v = vsb[:, 7:8]
            vsb2 = cp.tile([128, 1], FP)
            nc.sync.dma_start(out=vsb2[:, :], in_=biu[:, :])
            eps_t = cp.tile([128, 1], FP)
            nc.vector.memset(eps_t[:, :], EPS)
            zero_t = cp.tile([128, 1], FP)
            nc.vector.memset(zero_t[:, :], 0.0)

            natb_sb = cp.tile([128, 3 * F], odt)
            natb_ap = natb[:]
            natb_bcast = bass.AP(
                tensor=natb_ap.tensor, offset=natb_ap.offset,
                ap=[[0, 128]] + [list(d) for d in natb_ap.ap])
            nc.sync.dma_start(out=natb_sb[:, :], in_=natb_bcast)

            # ---- IO views ----
            x_view = x_in.rearrange("(p r) k -> p (r k)", p=128)    # [128, 4096]
            out_views = [o.rearrange("(p r) k -> p (r k)", p=128)
                         for o in (ga_out, ca_out, un_out)]

            sh_all = pp.tile([128, 2 * NT * F], ddt)               # [128, 8192]

            def shA(t):
                return sh_all[:, t * F:(t + 1) * F]

            def shB(t):
                return sh_all[:, (NT + t) * F:(NT + t + 1) * F]

            rstdA = pp.tile([128, F], ddt)
            rstdB = pp.tile([128, F], ddt)

            # ================= sweep 1 =================
            with (
                tc.tile_pool(name="ps_ti", bufs=2, space="PSUM") as ps_ti,
                tc.tile_pool(name="ps_sh", bufs=2, space="PSUM") as ps_sh,
                tc.tile_pool(name="ps_st", bufs=1, space="PSUM") as ps_st,
                tc.tile_pool(name="s1", bufs=4) as s1,
            ):
                st_A = ps_st.tile([128, F], FP, tag="stA")
                st_B = ps_st.tile([128, F], FP, tag="stB")
                for t in range(NT):
                    xn = s1.tile([128, F], xdt, tag="xn")
                    nc.sync.dma_start(out=xn[:, :],
                                      in_=x_view[:, t * F:(t + 1) * F])
                    pi = ps_ti.tile([128, F], xdt, tag="pi")
                    for c in range(4):
                        nc.tensor.transpose(
                            pi[:, c * 128:(c + 1) * 128],
                            xn[:, c * 128:(c + 1) * 128],
                            L_IDD)
                    xblk = s1.tile([128, F], ddt, tag="xblk")
                    nc.scalar.copy(xblk[:, :], pi[:, :])
                    psA = ps_sh.tile([128, F], pdt, tag="ps")
                    nc.tensor.matmul(psA[:, :], _mm(L_MA), _mm(xblk[:, :]))
                    nc.vector.tensor_scalar_add(shA(t), psA[:, :], cc1_v)
                    sqA = s1.tile([128, F], ddt, tag="sqA")
                    nc.gpsimd.tensor_mul(sqA[:, :], shA(t), shA(t))
                    nc.tensor.matmul(st_A[:, :], _mm(L_SP[t]), _mm(sqA[:, :]),
                                     start=(t == 0), stop=(t == NT - 1))
                    psB = ps_sh.tile([128, F], pdt, tag="ps")
                    nc.tensor.matmul(psB[:, :], _mm(L_MB), _mm(xblk[:, :]))
                    nc.vector.tensor_scalar_add(shB(t), psB[:, :], wB_v)
                    sqB = s1.tile([128, F], ddt, tag="sqB")
                    nc.gpsimd.tensor_mul(sqB[:, :], shB(t), shB(t))
                    nc.tensor.matmul(st_B[:, :], _mm(L_SP[t]), _mm(sqB[:, :]),
                                     start=(t == 0), stop=(t == NT - 1))

                # rstd = (sumsq/8 + eps)^-1/2
                if RSTD_MODE == "ars":
                    nc.scalar.activation(rstdA[:, :], st_A[:, :],
                                         AF.Abs_reciprocal_sqrt,
                                         bias=eps_t[:, 0:1], scale=1.0 / K)
                    nc.scalar.activation(rstdB[:, :], st_B[:, :],
                                         AF.Abs_reciprocal_sqrt,
                                         bias=eps_t[:, 0:1], scale=1.0 / K)
                else:
                    lt = s1.tile([128, F], FP, tag="lntmp")
                    nc.scalar.activation(lt[:, :], st_A[:, :], AF.Ln,
                                         bias=eps_t[:, 0:1], scale=1.0 / K)
                    nc.scalar.activation(rstdA[:, :], lt[:, :], AF.Exp,
                                         bias=zero_t[:, 0:1], scale=-0.5)
                    lt2 = s1.tile([128, F], FP, tag="lntmp")
                    nc.scalar.activation(lt2[:, :], st_B[:, :], AF.Ln,
                                         bias=eps_t[:, 0:1], scale=1.0 / K)
                    nc.scalar.activation(rstdB[:, :], lt2[:, :], AF.Exp,
                                         bias=zero_t[:, 0:1], scale=-0.5)

            # ================= sweep 2 =================
            with (
                tc.tile_pool(name="ps_bc", bufs=1, space="PSUM") as ps_bc,
                tc.tile_pool(name="ps_h", bufs=2, space="PSUM") as ps_h,
                tc.tile_pool(name="ps_f", bufs=3, space="PSUM") as ps_f,
                tc.tile_pool(name="ps_o", bufs=2, space="PSUM") as ps_o,
                tc.tile_pool(name="s2", bufs=3) as s2,
                tc.tile_pool(name="s2h", bufs=6) as s2h,
                tc.tile_pool(name="s2n", bufs=6) as s2n,
            ):
                for t in range(NT):
                    for path in ("A", "B"):
                        rstd = rstdA if path == "A" else rstdB
                        sh = shA(t) if path == "A" else shB(t)
                        g_v = g1_v if path == "A" else g2_v
                        L1t = L_W1A_t if path == "A" else L_W1B_t
                        L1b = L_W1A_b if path == "A" else L_W1B_b
                        L2t = L_W2A_t if path == "A" else L_W2B_t
                        L2b = L_W2A_b if path == "A" else L_W2B_b
                        bt_v = b1At_v if path == "A" else b1Bt_v
                        bb_v = b1Ab_v if path == "A" else b1Bb_v

                        bc = ps_bc.tile([128, F], pdt, tag="bc")
                        nc.tensor.matmul(bc[:, :], _mm(L_BCt[t]),
                                         _mm(rstd[:, :]))
                        z = s2.tile([128, F], ddt, tag="z" + path)
                        nc.vector.tensor_mul(z[:, :], sh, bc[:, :])
                        ph1 = ps_h.tile([128, F], pdt, tag="ph")
                        nc.tensor.matmul(ph1[:, :], _mm(L1t), _mm(z[:, :]))
                        h1 = s2h.tile([128, F], ddt, tag="h")
                        if BF and path == "B":
                            nc.vector.tensor_scalar(
                                out=h1[:, :], in0=ph1[:, :], scalar1=bt_v,
                                scalar2=0.0, op0=OP.add, op1=OP.max)
                        else:
                            nc.scalar.activation(h1[:, :], ph1[:, :], AF.Relu,
                                                 bias=bt_v, scale=1.0)
                        ph2 = ps_h.tile([128, F], pdt, tag="ph")
                        nc.tensor.matmul(ph2[:, :], _mm(L1b), _mm(z[:, :]))
                        h2 = s2h.tile([128, F], ddt, tag="h")
                        if BF and path == "B":
                            nc.vector.tensor_scalar(
                                out=h2[:, :], in0=ph2[:, :], scalar1=bb_v,
                                scalar2=0.0, op0=OP.add, op1=OP.max)
                        else:
                            nc.scalar.activation(h2[:, :], ph2[:, :], AF.Relu,
                                                 bias=bb_v, scale=1.0)
                        pf = ps_f.tile([128, F], pdt, tag="pf")
                        nc.tensor.matmul(pf[:, :], _mm(L2t), _mm(h1[:, :]),
                                         start=True, stop=False)
                        nc.tensor.matmul(pf[:, :], _mm(L2b), _mm(h2[:, :]),
                                         start=False, stop=True)
                        res = s2.tile([128, F], ddt, tag="res" + path)
                        nc.vector.scalar_tensor_tensor(
                            res[:, :], in0=z[:, :], scalar=g_v, in1=pf[:, :],
                            op0=OP.mult, op1=OP.add)
                        if path == "A":
                            resA = res
                        else:
                            resB = res

                    pu = ps_f.tile([128, F], pdt, tag="pf")
                    nc.tensor.matmul(pu[:, :], _mm(L_UA), _mm(resA[:, :]),
                                     start=True, stop=False)
                    nc.tensor.matmul(pu[:, :], _mm(L_UB), _mm(resB[:, :]),
                                     start=False, stop=True)
                    ublk = s2.tile([128, F], ddt, tag="ublk")
                    nc.scalar.activation(ublk[:, :], pu[:, :], AF.Tanh,
                                         bias=vsb2[:, 0:1], scale=1.0)

                    for r, srcb in ((0, resA), (1, resB), (2, ublk)):
                        po = ps_o.tile([128, F], ddt, tag="po")
                        for c in range(4):
                            sl = slice(c * 128, (c + 1) * 128)
                            nc.tensor.transpose(po[:, c * 128:(c + 1) * 128],
                                                srcb[:, sl], L_IDD)
                        natt = s2n.tile([128, F], odt, tag="natt")
                        if r == 2:
                            nc.scalar.copy(natt[:, :], po[:, :])
                        else:
                            nc.vector.tensor_add(
                                natt[:, :], po[:, :],
                                natb_sb[:, r * F:(r + 1) * F])
                        nc.sync.dma_start(
                            out=out_views[r][:, t * F:(t + 1) * F],
                            in_=natt[:, :])


    _split_multi_waits(nc)
    _NC_CACHE[key] = nc
    return nc


def _fold_constants(coral_taus, params):
    """Host-side constant folding in float64."""
    p = {k: {kk: np.asarray(vv, np.float64) for kk, vv in v.items()}
         if isinstance(v, dict) else np.asarray(v, np.float64)
         for k, v in params.items()}
    taus = np.asarray(coral_taus, np.float64)

    a1, a2 = p['attn1'], p['attn2']
    c1 = (taus @ a1['wv'].T + a1['bv']) @ a1['wo'].T + a1['bo']       # [8]
    A2 = a2['wv'].T @ a2['wo'].T                                       # [8,8] right-mult
    c2v = a2['bv'] @ a2['wo'].T + a2['bo'] + taus                      # [8]

    C = np.eye(K) - np.ones((K, K)) / K
    M_A = C
    M_B = A2 @ C
    cc1 = c1 @ C
    wB = c2v @ C

    def ffn_fold(fp, g, b):
        w1, b1, w2, b2 = fp['w1'], fp['b1'], fp['w2'], fp['b2']
        W1eff = (w1 * g[None, :]).T            # [8,16]: W1eff[k,j] = g[k] w1[j,k]
        b1eff = b @ w1.T + b1                  # [16]
        W2eff = w2.T                           # [16,8]
        bb = b + b2                            # [8]
        return W1eff, b1eff, W2eff, bb

    g1, b1v = p['ln1_g'], p['ln1_b']
    g2, b2v = p['ln2_g'], p['ln2_b']
    W1A, b1A, W2A, bbA = ffn_fold(p['ffn1'], g1, b1v)
    W1B, b1B, W2B, bbB = ffn_fold(p['ffn2'], g2, b2v)

    int_w, int_b = p['int_w'], p['int_b']
    Wa = int_w[:, :K].T                        # [8,8]
    Wb = int_w[:, K:].T
    biU = int_b + bbA @ Wa + bbB @ Wb

    I16 = np.eye(16)

    def kr(M):
        return np.kron(I16, M)

    mats = [kr(M_A), kr(M_B),
            kr(W1A[:, 0:8]), kr(W1A[:, 8:16]),
            kr(W1B[:, 0:8]), kr(W1B[:, 8:16]),
            kr(W2A[0:8, :]), kr(W2A[8:16, :]),
            kr(W2B[0:8, :]), kr(W2B[8:16, :]),
            kr(Wa), kr(Wb)]
    nt = RPC // (16 * F)
    ones16 = np.kron(I16, np.ones((8, 1)))     # [128, 16]
    for t in range(nt):                        # stat spread: sums land at 16t+g
        sp = np.zeros((128, 128))
        sp[:, 16 * t:16 * t + 16] = ones16
        mats.append(sp)
    for t in range(nt):                        # rstd bcast: out (g,k) <- rstd[16t+g]
        bc = np.zeros((128, 128))
        bc[16 * t:16 * t + 16, :] = np.kron(I16, np.ones((1, 8)))
        mats.append(bc)
    mats.append(np.eye(128))                   # identity for transposes
    wpack = np.concatenate(mats, axis=1)

    def t16(v):
        return np.tile(v, 16)

    vpack = np.stack([t16(cc1), t16(wB), t16(g1), t16(g2),
                      t16(b1A[0:8]), t16(b1A[8:16]),
                      t16(b1B[0:8]), t16(b1B[8:16])], axis=1)   # [128, 8]

    natb = np.concatenate([np.tile(bbA, 64), np.tile(bbB, 64),
                           np.zeros(F)])                          # [1536]

    biU_t = t16(biU)                                              # [128]
    return (wpack.astype(np.float32), vpack.astype(np.float32),
            natb.astype(np.float32), biU_t.astype(np.float32))


_RUNNER_CACHE = {}


def _make_runner(nc):
    """Cached jitted shard_map executable over the 8 cores (no donation so
    it can be re-invoked for timing)."""
    if id(nc) in _RUNNER_CACHE:
        return _RUNNER_CACHE[id(nc)]
    import jax
    from jax.experimental.shard_map import shard_map
    from jax.sharding import Mesh, PartitionSpec
    from concourse import bass2jax
    import concourse.mybir as _mybir

    bass2jax.install_neuronx_cc_hook()
    in_names, out_names, out_avals, zero_shapes = [], [], [], []
    for alloc in nc.m.functions[0].allocations:
        if not isinstance(_mybir.MemoryLocationSet, type) or not isinstance(
                alloc, _mybir.MemoryLocationSet):
            continue
        name = alloc.memorylocations[0].name
        pname = (nc.partition_id_tensor.name
                 if nc.partition_id_tensor else None)
        if alloc.kind == "ExternalInput":
            if name != pname:
                in_names.append(name)
        elif alloc.kind == "ExternalOutput":
            out_names.append(name)
            shape = tuple(alloc.tensor_shape)
            dtype = _mybir.dt.np(alloc.dtype)
            out_avals.append(jax.core.ShapedArray(shape, dtype))
            zero_shapes.append((shape, dtype))
    n_params = len(in_names)
    all_names = list(in_names) + list(out_names)
    if nc.partition_id_tensor is not None:
        all_names.append(nc.partition_id_tensor.name)

    def _body(*args):
        operands = list(args)
        if nc.partition_id_tensor is not None:
            operands.append(bass2jax.partition_id_tensor())
        outs = bass2jax._bass_exec_p.bind(
            *operands,
            out_avals=tuple(out_avals),
            in_names=tuple(all_names),
            out_names=tuple(out_names),
            lowering_input_output_aliases=(),
            sim_require_finite=True,
            sim_require_nnan=True,
            nc=nc)
        return tuple(outs)

    devices = jax.devices()[:NCORES]
    mesh = Mesh(np.asarray(devices), ("core",))
    n_args = n_params + len(out_names)
    fn = jax.jit(shard_map(
        _body, mesh=mesh,
        in_specs=(PartitionSpec("core"),) * n_args,
        out_specs=(PartitionSpec("core"),) * len(out_names),
        check_rep=False))
    runner = (fn, in_names, out_names, zero_shapes)
    _RUNNER_CACHE[id(nc)] = runner
    return runner


def _run(nc, in_maps):
    fn, in_names, out_names, zero_shapes = _make_runner(nc)
    ncores = len(in_maps)
    concat_in = [np.concatenate([np.asarray(m[n]) for m in in_maps], axis=0)
                 for n in in_names]
    zeros = [np.zeros((ncores * s[0], *s[1:]), d) for s, d in zero_shapes]
    outs = fn(*concat_in, *zeros)
    results = []
    for c in range(ncores):
        results.append({
            name: np.asarray(outs[i]).reshape(ncores, *zero_shapes[i][0])[c]
            for i, name in enumerate(out_names)})
    return results


def _make_in_maps(gpcm_betas, coral_taus, params):
    x = np.asarray(gpcm_betas, np.float32).reshape(N, K)
    wpack, vpack, natb, biU_t = _fold_constants(coral_taus, params)
    if MM_MODE == "bf16":
        import ml_dtypes
        bt = ml_dtypes.bfloat16
        x = x.astype(bt)
        wpack = wpack.astype(bt)
        natb = natb.astype(bt)
    in_maps = []
    for i in range(NCORES):
        shard = np.ascontiguousarray(x[i * RPC:(i + 1) * RPC])
        in_maps.append({"x": shard, "wpack": wpack, "vpack": vpack,
                        "natb": natb, "biu": biU_t.reshape(128, 1)})
    return in_maps


def bench(inputs, reps=20):
    """Min wall time per executable invocation (device exec + dispatch)."""
    import time as _time
    import jax
    nc = _build_nc()
    in_maps = _make_in_maps(inputs["gpcm_betas"], inputs["coral_taus"],
                            inputs["params"])
    fn, in_names, out_names, zero_shapes = _make_runner(nc)
    ncores = len(in_maps)
    concat_in = [np.concatenate([np.asarray(m[n]) for m in in_maps], axis=0)
                 for n in in_names]
    zeros = [np.zeros((ncores * s[0], *s[1:]), d) for s, d in zero_shapes]
    args = [jax.device_put(a) for a in concat_in + zeros]
    best = float("inf")
    for _ in range(reps):
        t0 = _time.perf_counter()
        outs = fn(*args)
        jax.block_until_ready(outs)
        dt = _time.perf_counter() - t0
        best = min(best, dt)
    return best * 1e9


def kernel(gpcm_betas, coral_taus, theta, params):
    nc = _build_nc()
    in_maps = _make_in_maps(gpcm_betas, coral_taus, params)
    results = _run(nc, in_maps)

    ga = np.concatenate([results[i]["ga"] for i in range(NCORES)], axis=0)
    ca = np.concatenate([results[i]["ca"] for i in range(NCORES)], axis=0)
    un = np.concatenate([results[i]["un"] for i in range(NCORES)], axis=0)
    ga = ga.astype(np.float32)
    ca = ca.astype(np.float32)
    un = un.astype(np.float32)

    ones = np.ones((N, 1), np.float32)
    return (un.reshape(B, S, K), ga.reshape(B, S, K), ca.reshape(B, S, K),
            ones, ones.copy())


# revision 33
# speedup vs baseline: 1.0702x; 1.0574x over previous
"""Trainium2 Bass kernel for nn_AttentionThresholdCoupler.

Math notes (derived from the reference):
  - The cross-attention has query/key sequence length 1, so softmax over the
    single key is exactly 1.0: attention weights gw/cw are all-ones and the
    attention blocks collapse to affine maps.
  - Path A (ga): LN(x + c1) with c1 a constant vector, then FFN residual.
  - Path B (ca): LN(x @ A2 + c2) with A2 = wv2.T @ wo2.T, then FFN residual.
  - unified = tanh(ga @ Wa + ca @ Wb + bi).
All per-row ops act on K=8 features; rows are B*S = 524288, data-parallel
across 8 cores (65536 rows each).

Device layout: "blk" layout puts K on partitions: a tile [128, 512] holds 16
row-groups x 8 features on partitions, 512 row-slots on the free dim.  PE
transposes [128,128] chunks convert between the HBM-natural layout (rows on
partitions) and blk.  All small matmuls become block-diagonal [128,128]
matmuls (np.kron(I16, M)).
"""

import sys

if '/opt/trn_rl_repo' not in sys.path:
    sys.path.insert(0, '/opt/trn_rl_repo')

import numpy as np

import concourse.bass as bass
import concourse.mybir as mybir
import concourse.tile as tile
from concourse.vector_clock import ScopedClock, VectorClock

K = 8
B = 256
S = 2048
N = B * S
NCORES = 8
RPC = N // NCORES          # rows per core = 65536
F = 512                    # free dim per X-tile (one PSUM bank of fp32)
NT = RPC // (16 * F)       # X-tiles per core = 8
EPS = 1e-5

FP = mybir.dt.float32
AF = mybir.ActivationFunctionType
OP = mybir.AluOpType

# matmul operand mode: "f32" | "f32r" | "bf16"
MM_MODE = "f32r"
# rstd mode: "ars" (Abs_reciprocal_sqrt, 1 op) | "lnexp" (Ln+Exp, 2 ops)
RSTD_MODE = "lnexp"


def _patch_tile_drain():
    """walrus in this container rejects >1 sync waits on the Tile exit
    drain; split them across one drain instruction per proc."""
    if getattr(tile.TileContext, '_drain_split_patched', False):
        return

    def _drain_and_barrier_split(self, tick_clock, wait_clock):
        nc = self.nc
        vclock = tick_clock.global_clock
        n = len(vclock)
        for i in range(n):
            t = vclock[i]
            if t > 0:
                partial = VectorClock([t if j == i else 0 for j in range(n)])
                d = nc.sync.drain()
                wait_clock.add_sem_waits(d.ins, ScopedClock({None: partial}))
        nc.sync.drain()
        nc.all_engine_barrier()
        assert self.sems is not None
        popped = nc._tile_sem_poison_stack.pop()
        assert popped is self._sem_poison
        nc.clear_and_free_semaphores(list(self.sems.allocated().values()))
        nc.all_engine_barrier()

    tile.TileContext._drain_and_barrier = _drain_and_barrier_split
    tile.TileContext._drain_split_patched = True


_WAIT_LIMIT = 1


def _split_multi_waits(nc):
    """walrus here rejects instructions carrying more than ~1 sync wait.
    Move excess waits onto same-engine NOPs inserted before the holder."""
    cnt = [0]
    for f in nc.m.functions:
        for blk in f.blocks:
            insts = blk.instructions
            out = []
            for inst in insts:
                si = inst.sync_info
                if si is not None and si.on_wait and len(si.on_wait) > _WAIT_LIMIT:
                    waits = list(si.on_wait)
                    keep = waits[-_WAIT_LIMIT:]
                    spill = waits[:-_WAIT_LIMIT]
                    for w in spill:
                        nop = mybir.InstNoOp(
                            name=f"waitnop-{cnt[0]}", ins=[], outs=[])
                        cnt[0] += 1
                        nop.engine = inst.engine
                        nop.sync_info = mybir.SyncInfo(
                            on_wait=[w], on_update=[])
                        out.append(nop)
                    inst.sync_info = mybir.SyncInfo(
                        on_wait=keep, on_update=list(si.on_update or []))
                out.append(inst)
            blk.instructions = out


def _mm_dt():
    return {"f32": mybir.dt.float32,
            "f32r": mybir.dt.float32r,
            "bf16": mybir.dt.bfloat16}[MM_MODE]


def _mm(ap):
    """Matmul operand APs are pre-typed via tile dtypes; no-op."""
    return ap


_NC_CACHE = {}


def _build_nc():
    """Build the Bass module (shape-only; weights arrive as inputs)."""
    key = (MM_MODE, RSTD_MODE)
    if key in _NC_CACHE:
        return _NC_CACHE[key]
    _patch_tile_drain()
    nc = bass.Bass()
    wdt = _mm_dt()
    ddt = _mm_dt()
    BF = MM_MODE == "bf16"
    BT = mybir.dt.bfloat16
    xdt = _mm_dt() if MM_MODE != "f32" else FP   # input pipeline dtype
    odt = BT if BF else FP          # output pipeline dtype
    pdt = FP                        # matmul outputs must be fp32

    x_in = nc.declare_dram_parameter("x", [RPC, K], xdt, isOutput=False)
    # (f32r container is np.float32; DMA bytes are unrounded fp32, PE rounds)
    # packed lhsT weights: 12 main + 8 stat-spread + 8 rstd-bcast + identity
    NW = 29
    wpack = nc.declare_dram_parameter("wpack", [128, NW * 128],
                                      _mm_dt() if MM_MODE != "f32" else FP,
                                      isOutput=False)
    vpack = nc.declare_dram_parameter("vpack", [128, 8], FP, isOutput=False)
    biu = nc.declare_dram_parameter("biu", [128, 1], FP, isOutput=False)
    natb = nc.declare_dram_parameter("natb", [3 * F], odt, isOutput=False)
    ga_out = nc.declare_dram_parameter("ga", [RPC, K], odt, isOutput=True)
    ca_out = nc.declare_dram_parameter("ca", [RPC, K], odt, isOutput=True)
    un_out = nc.declare_dram_parameter("un", [RPC, K], odt, isOutput=True)

    with tile.TileContext(nc) as tc:
        with (
            tc.tile_pool(name="consts", bufs=1) as cp,
            tc.tile_pool(name="persist", bufs=1) as pp,
        ):
            # ---- constants ----
            wsb = cp.tile([128, NW * 128], wdt)
            nc.sync.dma_start(out=wsb[:, :], in_=wpack[:, :])
            wsb_f32 = wsb

            def W(i):  # i-th [128,128] lhsT
                return wsb[:, i * 128:(i + 1) * 128]

            (L_MA, L_MB, L_W1A_t, L_W1A_b, L_W1B_t, L_W1B_b,
             L_W2A_t, L_W2A_b, L_W2B_t, L_W2B_b, L_UA, L_UB) = \
                [W(i) for i in range(12)]
            L_SP = [W(12 + i) for i in range(NT)]
            L_BCt = [W(20 + i) for i in range(NT)]
            # f32 identity for the input transposes (X_nat is f32);
            # dtype-matched identity for the output transposes
            L_IDF = wsb_f32[:, 28 * 128:29 * 128]
            L_IDD = wsb[:, 28 * 128:29 * 128]

            vsb = cp.tile([128, 8], FP)
            nc.sync.dma_start(out=vsb[:, :], in_=vpack[:, :])
            cc1_v = vsb[:, 0:1]
            wB_v = vsb[:, 1:2]
            g1_v = vsb[:, 2:3]
            g2_v = vsb[:, 3:4]
            b1At_v = vsb[:, 4:5]
            b1Ab_v = vsb[:, 5:6]
            # packed: 4=b1A top, 5=b1A bot, 6=b1B top, 7=b1B bot... need biU too
            b1Bt_v = vsb[:, 6:7]
            b1Bb_v = vsb[:, 7:8]
            vsb2 = cp.tile([128, 1], FP)
            nc.sync.dma_start(out=vsb2[:, :], in_=biu[:, :])
            eps_t = cp.tile([128, 1], FP)
            nc.vector.memset(eps_t[:, :], EPS)
            zero_t = cp.tile([128, 1], FP)
            nc.vector.memset(zero_t[:, :], 0.0)

            natb_sb = cp.tile([128, 3 * F], odt)
            natb_ap = natb[:]
            natb_bcast = bass.AP(
                tensor=natb_ap.tensor, offset=natb_ap.offset,
                ap=[[0, 128]] + [list(d) for d in natb_ap.ap])
            nc.sync.dma_start(out=natb_sb[:, :], in_=natb_bcast)

            # ---- IO views ----
            x_view = x_in.rearrange("(p r) k -> p (r k)", p=128)    # [128, 4096]
            out_views = [o.rearrange("(p r) k -> p (r k)", p=128)
                         for o in (ga_out, ca_out, un_out)]

            sh_all = pp.tile([128, 2 * NT * F], ddt)               # [128, 8192]

            def shA(t):
                return sh_all[:, t * F:(t + 1) * F]

            def shB(t):
                return sh_all[:, (NT + t) * F:(NT + t + 1) * F]

            rstdA = pp.tile([128, F], ddt)
            rstdB = pp.tile([128, F], ddt)

            # ================= sweep 1 =================
            with (
                tc.tile_pool(name="ps_ti", bufs=2, space="PSUM") as ps_ti,
                tc.tile_pool(name="ps_sh", bufs=2, space="PSUM") as ps_sh,
                tc.tile_pool(name="ps_st", bufs=1, space="PSUM") as ps_st,
                tc.tile_pool(name="s1", bufs=4) as s1,
            ):
                st_A = ps_st.tile([128, F], FP, tag="stA")
                st_B = ps_st.tile([128, F], FP, tag="stB")
                for t in range(NT):
                    xn = s1.tile([128, F], xdt, tag="xn")
                    nc.sync.dma_start(out=xn[:, :],
                                      in_=x_view[:, t * F:(t + 1) * F])
                    pi = ps_ti.tile([128, F], xdt, tag="pi")
                    for c in range(4):
                        nc.tensor.transpose(
                            pi[:, c * 128:(c + 1) * 128],
                            xn[:, c * 128:(c + 1) * 128],
                            L_IDD)
                    xblk = s1.tile([128, F], ddt, tag="xblk")
                    nc.scalar.copy(xblk[:, :], pi[:, :])
                    psA = ps_sh.tile([128, F], pdt, tag="ps")
                    nc.tensor.matmul(psA[:, :], _mm(L_MA), _mm(xblk[:, :]))
                    nc.vector.tensor_scalar_add(shA(t), psA[:, :], cc1_v)
                    sqA = s1.tile([128, F], ddt, tag="sqA")
                    nc.gpsimd.tensor_mul(sqA[:, :], shA(t), shA(t))
                    nc.tensor.matmul(st_A[:, :], _mm(L_SP[t]), _mm(sqA[:, :]),
                                     start=(t == 0), stop=(t == NT - 1))
                    psB = ps_sh.tile([128, F], pdt, tag="ps")
                    nc.tensor.matmul(psB[:, :], _mm(L_MB), _mm(xblk[:, :]))
                    nc.vector.tensor_scalar_add(shB(t), psB[:, :], wB_v)
                    sqB = s1.tile([128, F], ddt, tag="sqB")
                    nc.gpsimd.tensor_mul(sqB[:, :], shB(t), shB(t))
                    nc.tensor.matmul(st_B[:, :], _mm(L_SP[t]), _mm(sqB[:, :]),
                                     start=(t == 0), stop=(t == NT - 1))

                # rstd = (sumsq/8 + eps)^-1/2
                if RSTD_MODE == "ars":
                    nc.scalar.activation(rstdA[:, :], st_A[:, :],
                                         AF.Abs_reciprocal_sqrt,
                                         bias=eps_t[:, 0:1], scale=1.0 / K)
                    nc.scalar.activation(rstdB[:, :], st_B[:, :],
                                         AF.Abs_reciprocal_sqrt,
                                         bias=eps_t[:, 0:1], scale=1.0 / K)
                else:
                    lt = s1.tile([128, F], FP, tag="lntmp")
                    nc.scalar.activation(lt[:, :], st_A[:, :], AF.Ln,
                                         bias=eps_t[:, 0:1], scale=1.0 / K)
                    nc.scalar.activation(rstdA[:, :], lt[:, :], AF.Exp,
                                         bias=zero_t[:, 0:1], scale=-0.5)
                    lt2 = s1.tile([128, F], FP, tag="lntmp")
                    nc.scalar.activation(lt2[:, :], st_B[:, :], AF.Ln,
                                         bias=eps_t[:, 0:1], scale=1.0 / K)
                    nc.scalar.activation(rstdB[:, :], lt2[:, :], AF.Exp,
                                         bias=zero_t[:, 0:1], scale=-0.5)

            # ================= sweep 2 =================
            with (
                tc.tile_pool(name="ps_bc", bufs=1, space="PSUM") as ps_bc,
                tc.tile_pool(name="ps_h", bufs=2, space="PSUM") as ps_h,
                tc.tile_pool(name="ps_f", bufs=3, space="PSUM") as ps_f,
                tc.tile_pool(name="ps_o", bufs=2, space="PSUM") as ps_o,
                tc.tile_pool(name="s2", bufs=3) as s2,
                tc.tile_pool(name="s2h", bufs=6) as s2h,
                tc.tile_pool(name="s2n", bufs=6) as s2n,
            ):
                for t in range(NT):
                    for path in ("A", "B"):
                        rstd = rstdA if path == "A" else rstdB
                        sh = shA(t) if path == "A" else shB(t)
                        g_v = g1_v if path == "A" else g2_v
                        L1t = L_W1A_t if path == "A" else L_W1B_t
                        L1b = L_W1A_b if path == "A" else L_W1B_b
                        L2t = L_W2A_t if path == "A" else L_W2B_t
                        L2b = L_W2A_b if path == "A" else L_W2B_b
                        bt_v = b1At_v if path == "A" else b1Bt_v
                        bb_v = b1Ab_v if path == "A" else b1Bb_v

                        bc = ps_bc.tile([128, F], pdt, tag="bc")
                        nc.tensor.matmul(bc[:, :], _mm(L_BCt[t]),
                                         _mm(rstd[:, :]))
                        z = s2.tile([128, F], ddt, tag="z" + path)
                        nc.vector.tensor_mul(z[:, :], sh, bc[:, :])
                        ph1 = ps_h.tile([128, F], pdt, tag="ph")
                        nc.tensor.matmul(ph1[:, :], _mm(L1t), _mm(z[:, :]))
                        h1 = s2h.tile([128, F], ddt, tag="h")
                        if BF and path == "B":
                            nc.vector.tensor_scalar(
                                out=h1[:, :], in0=ph1[:, :], scalar1=bt_v,
                                scalar2=0.0, op0=OP.add, op1=OP.max)
                        else:
                            nc.scalar.activation(h1[:, :], ph1[:, :], AF.Relu,
                                                 bias=bt_v, scale=1.0)
                        ph2 = ps_h.tile([128, F], pdt, tag="ph")
                        nc.tensor.matmul(ph2[:, :], _mm(L1b), _mm(z[:, :]))
                        h2 = s2h.tile([128, F], ddt, tag="h")
                        if BF and path == "B":
                            nc.vector.tensor_scalar(
                                out=h2[:, :], in0=ph2[:, :], scalar1=bb_v,
                                scalar2=0.0, op0=OP.add, op1=OP.max)
                        else:
                            nc.scalar.activation(h2[:, :], ph2[:, :], AF.Relu,
                                                 bias=bb_v, scale=1.0)
                        pf = ps_f.tile([128, F], pdt, tag="pf")
                        nc.tensor.matmul(pf[:, :], _mm(L2t), _mm(h1[:, :]),
                                         start=True, stop=False)
                        nc.tensor.matmul(pf[:, :], _mm(L2b), _mm(h2[:, :]),
                                         start=False, stop=True)
                        res = s2.tile([128, F], ddt, tag="res" + path)
                        nc.vector.scalar_tensor_tensor(
                            res[:, :], in0=z[:, :], scalar=g_v, in1=pf[:, :],
                            op0=OP.mult, op1=OP.add)
                        if path == "A":
                            resA = res
                        else:
                            resB = res

                    pu = ps_f.tile([128, F], pdt, tag="pf")
                    nc.tensor.matmul(pu[:, :], _mm(L_UA), _mm(resA[:, :]),
                                     start=True, stop=False)
                    nc.tensor.matmul(pu[:, :], _mm(L_UB), _mm(resB[:, :]),
                                     start=False, stop=True)
                    ublk = s2.tile([128, F], ddt, tag="ublk")
                    nc.scalar.activation(ublk[:, :], pu[:, :], AF.Tanh,
                                         bias=vsb2[:, 0:1], scale=1.0)

                    for r, srcb in ((0, resA), (1, resB), (2, ublk)):
                        po = ps_o.tile([128, F], ddt, tag="po")
                        for c in range(4):
                            sl = slice(c * 128, (c + 1) * 128)
                            nc.tensor.transpose(po[:, c * 128:(c + 1) * 128],
                                                srcb[:, sl], L_IDD)
                        natt = s2n.tile([128, F], odt, tag="natt")
                        if r == 2:
                            nc.scalar.copy(natt[:, :], po[:, :])
                        else:
                            nc.vector.tensor_add(
                                natt[:, :], po[:, :],
                                natb_sb[:, r * F:(r + 1) * F])
                        nc.sync.dma_start(
                            out=out_views[r][:, t * F:(t + 1) * F],
                            in_=natt[:, :])


    _split_multi_waits(nc)
    _NC_CACHE[key] = nc
    return nc


def _fold_constants(coral_taus, params):
    """Host-side constant folding in float64."""
    p = {k: {kk: np.asarray(vv, np.float64) for kk, vv in v.items()}
         if isinstance(v, dict) else np.asarray(v, np.float64)
         for k, v in params.items()}
    taus = np.asarray(coral_taus, np.float64)

    a1, a2 = p['attn1'], p['attn2']
    c1 = (taus @ a1['wv'].T + a1['bv']) @ a1['wo'].T + a1['bo']       # [8]
    A2 = a2['wv'].T @ a2['wo'].T                                       # [8,8] right-mult
    c2v = a2['bv'] @ a2['wo'].T + a2['bo'] + taus                      # [8]

    C = np.eye(K) - np.ones((K, K)) / K
    M_A = C
    M_B = A2 @ C
    cc1 = c1 @ C
    wB = c2v @ C

    def ffn_fold(fp, g, b):
        w1, b1, w2, b2 = fp['w1'], fp['b1'], fp['w2'], fp['b2']
        W1eff = (w1 * g[None, :]).T            # [8,16]: W1eff[k,j] = g[k] w1[j,k]
        b1eff = b @ w1.T + b1                  # [16]
        W2eff = w2.T                           # [16,8]
        bb = b + b2                            # [8]
        return W1eff, b1eff, W2eff, bb

    g1, b1v = p['ln1_g'], p['ln1_b']
    g2, b2v = p['ln2_g'], p['ln2_b']
    W1A, b1A, W2A, bbA = ffn_fold(p['ffn1'], g1, b1v)
    W1B, b1B, W2B, bbB = ffn_fold(p['ffn2'], g2, b2v)

    int_w, int_b = p['int_w'], p['int_b']
    Wa = int_w[:, :K].T                        # [8,8]
    Wb = int_w[:, K:].T
    biU = int_b + bbA @ Wa + bbB @ Wb

    I16 = np.eye(16)

    def kr(M):
        return np.kron(I16, M)

    mats = [kr(M_A), kr(M_B),
            kr(W1A[:, 0:8]), kr(W1A[:, 8:16]),
            kr(W1B[:, 0:8]), kr(W1B[:, 8:16]),
            kr(W2A[0:8, :]), kr(W2A[8:16, :]),
            kr(W2B[0:8, :]), kr(W2B[8:16, :]),
            kr(Wa), kr(Wb)]
    nt = RPC // (16 * F)
    ones16 = np.kron(I16, np.ones((8, 1)))     # [128, 16]
    for t in range(nt):                        # stat spread: sums land at 16t+g
        sp = np.zeros((128, 128))
        sp[:, 16 * t:16 * t + 16] = ones16
        mats.append(sp)
    for t in range(nt):                        # rstd bcast: out (g,k) <- rstd[16t+g]
        bc = np.zeros((128, 128))
        bc[16 * t:16 * t + 16, :] = np.kron(I16, np.ones((1, 8)))
        mats.append(bc)
    mats.append(np.eye(128))                   # identity for transposes
    wpack = np.concatenate(mats, axis=1)

    def t16(v):
        return np.tile(v, 16)

    vpack = np.stack([t16(cc1), t16(wB), t16(g1), t16(g2),
                      t16(b1A[0:8]), t16(b1A[8:16]),
                      t16(b1B[0:8]), t16(b1B[8:16])], axis=1)   # [128, 8]

    natb = np.concatenate([np.tile(bbA, 64), np.tile(bbB, 64),
                           np.zeros(F)])                          # [1536]

    biU_t = t16(biU)                                              # [128]
    return (wpack.astype(np.float32), vpack.astype(np.float32),
            natb.astype(np.float32), biU_t.astype(np.float32))


_RUNNER_CACHE = {}


def _make_runner(nc):
    """Cached jitted shard_map executable over the 8 cores (no donation so
    it can be re-invoked for timing)."""
    if id(nc) in _RUNNER_CACHE:
        return _RUNNER_CACHE[id(nc)]
    import jax
    from jax.experimental.shard_map import shard_map
    from jax.sharding import Mesh, PartitionSpec
    from concourse import bass2jax
    import concourse.mybir as _mybir

    bass2jax.install_neuronx_cc_hook()
    in_names, out_names, out_avals, zero_shapes = [], [], [], []
    for alloc in nc.m.functions[0].allocations:
        if not isinstance(_mybir.MemoryLocationSet, type) or not isinstance(
                alloc, _mybir.MemoryLocationSet):
            continue
        name = alloc.memorylocations[0].name
        pname = (nc.partition_id_tensor.name
                 if nc.partition_id_tensor else None)
        if alloc.kind == "ExternalInput":
            if name != pname:
                in_names.append(name)
        elif alloc.kind == "ExternalOutput":
            out_names.append(name)
            shape = tuple(alloc.tensor_shape)
            dtype = _mybir.dt.np(alloc.dtype)
            out_avals.append(jax.core.ShapedArray(shape, dtype))
            zero_shapes.append((shape, dtype))
    n_params = len(in_names)
    all_names = list(in_names) + list(out_names)
    if nc.partition_id_tensor is not None:
        all_names.append(nc.partition_id_tensor.name)

    def _body(*args):
        operands = list(args)
        if nc.partition_id_tensor is not None:
            operands.append(bass2jax.partition_id_tensor())
        outs = bass2jax._bass_exec_p.bind(
            *operands,
            out_avals=tuple(out_avals),
            in_names=tuple(all_names),
            out_names=tuple(out_names),
            lowering_input_output_aliases=(),
            sim_require_finite=True,
            sim_require_nnan=True,
            nc=nc)
        return tuple(outs)

    devices = jax.devices()[:NCORES]
    mesh = Mesh(np.asarray(devices), ("core",))
    n_args = n_params + len(out_names)
    fn = jax.jit(shard_map(
        _body, mesh=mesh,
        in_specs=(PartitionSpec("core"),) * n_args,
        out_specs=(PartitionSpec("core"),) * len(out_names),
        check_rep=False))
    runner = (fn, in_names, out_names, zero_shapes)
    _RUNNER_CACHE[id(nc)] = runner
    return runner


def _run(nc, in_maps):
    fn, in_names, out_names, zero_shapes = _make_runner(nc)
    ncores = len(in_maps)
    concat_in = [np.concatenate([np.asarray(m[n]) for m in in_maps], axis=0)
                 for n in in_names]
    zeros = [np.zeros((ncores * s[0], *s[1:]), d) for s, d in zero_shapes]
    outs = fn(*concat_in, *zeros)
    results = []
    for c in range(ncores):
        results.append({
            name: np.asarray(outs[i]).reshape(ncores, *zero_shapes[i][0])[c]
            for i, name in enumerate(out_names)})
    return results


def _make_in_maps(gpcm_betas, coral_taus, params):
    x = np.asarray(gpcm_betas, np.float32).reshape(N, K)
    wpack, vpack, natb, biU_t = _fold_constants(coral_taus, params)
    if MM_MODE == "bf16":
        import ml_dtypes
        bt = ml_dtypes.bfloat16
        x = x.astype(bt)
        wpack = wpack.astype(bt)
        natb = natb.astype(bt)
    in_maps = []
    for i in range(NCORES):
        shard = np.ascontiguousarray(x[i * RPC:(i + 1) * RPC])
        in_maps.append({"x": shard, "wpack": wpack, "vpack": vpack,
                        "natb": natb, "biu": biU_t.reshape(128, 1)})
    return in_maps


def bench(inputs, reps=20):
    """Min wall time per executable invocation (device exec + dispatch)."""
    import time as _time
    import jax
    nc = _build_nc()
    in_maps = _make_in_maps(inputs["gpcm_betas"], inputs["coral_taus"],
                            inputs["params"])
    fn, in_names, out_names, zero_shapes = _make_runner(nc)
    ncores = len(in_maps)
    concat_in = [np.concatenate([np.asarray(m[n]) for m in in_maps], axis=0)
                 for n in in_names]
    zeros = [np.zeros((ncores * s[0], *s[1:]), d) for s, d in zero_shapes]
    args = [jax.device_put(a) for a in concat_in + zeros]
    best = float("inf")
    for _ in range(reps):
        t0 = _time.perf_counter()
        outs = fn(*args)
        jax.block_until_ready(outs)
        dt = _time.perf_counter() - t0
        best = min(best, dt)
    return best * 1e9


def kernel(gpcm_betas, coral_taus, theta, params):
    nc = _build_nc()
    in_maps = _make_in_maps(gpcm_betas, coral_taus, params)
    results = _run(nc, in_maps)

    ga = np.concatenate([results[i]["ga"] for i in range(NCORES)], axis=0)
    ca = np.concatenate([results[i]["ca"] for i in range(NCORES)], axis=0)
    un = np.concatenate([results[i]["un"] for i in range(NCORES)], axis=0)
    ga = ga.astype(np.float32)
    ca = ca.astype(np.float32)
    un = un.astype(np.float32)

    ones = np.ones((N, 1), np.float32)
    return (un.reshape(B, S, K), ga.reshape(B, S, K), ca.reshape(B, S, K),
            ones, ones.copy())
